# revision 10
# baseline (speedup 1.0000x reference)
"""AMPBlock0 (BigVGAN) Trainium2 kernel: B=8 data-parallel over 8 NeuronCores.

Per core: x (512, 8192) f32 -> out (512, 8188) f32
  a1 = down1(snake1(up1(x)))       # polyphase up x2, SnakeBeta, stride-2 lowpass
  c1 = conv1d_3tap(a1) + b1
  a2 = down2(snake2(up2(c1)))
  out = conv1d_3tap(a2) + b2 + x[:, :8188]

Layout: channels on partitions (4 blocks x 128), time on free axis.
Time tiled (L=1024) with halos. bf16 storage/matmul, f32 PSUM.
Engine split (v2): PE = dense convs + 12-tap down convs (diag matmuls);
DVE = up-conv middle taps (STT chains); ACT = first up-tap (Identity w/
per-channel scale+bias), cos via Sin(scale*acc + bias'), psum evicts;
Pool = snake combine (mul-bcast + add), last up-tap, residual add.
"""

import sys

if "/opt/trn_rl_repo" not in sys.path:
    sys.path.insert(0, "/opt/trn_rl_repo")

import numpy as np
import ml_dtypes

import concourse.bacc as bacc
import concourse.mybir as mybir
import concourse.tile as tile
from concourse.bass_utils import run_bass_kernel_spmd

BF16 = mybir.dt.bfloat16
F32 = mybir.dt.float32
AF = mybir.ActivationFunctionType
ALU = mybir.AluOpType

T = 8192
C = 512
NB = 4
L = 1024
NT = T // L
PAD = 16
TOUT = T - 4
CHUNK = 512

# sc columns (128, NB, 64) f32; stage offset S2=28
# 0-5 we, 6-11 wo, 12-17 d_o, 18-23 d_e, 24 scaleA, 25 biasS, 26 ninv2b, 27 inv2b
# 56 bias1(cout), 57 bias2(cout)
S2 = 28

LAST_EXEC_NS = None
LAST_PROFILE = None


def _chunks(width):
    out, c0 = [], 0
    while c0 < width:
        out.append((c0, min(CHUNK, width - c0)))
        c0 += CHUNK
    return out


def build_graph():
    nc = bacc.Bacc()
    xp_d = nc.declare_dram_parameter("xp", [128, NB, T + 2 * PAD], BF16, isOutput=False)
    w1t_d = nc.declare_dram_parameter("w1t", [128, 3, NB, NB, 128], BF16, isOutput=False)
    w2t_d = nc.declare_dram_parameter("w2t", [128, 3, NB, NB, 128], BF16, isOutput=False)
    sc_d = nc.declare_dram_parameter("sc", [128, NB, 64], F32, isOutput=False)
    diag_d = nc.declare_dram_parameter("diag", [128, 2 * NB * 12, 128], BF16, isOutput=False)
    out_d = nc.declare_dram_parameter("out", [128, NB, TOUT], BF16, isOutput=True)

    with tile.TileContext(nc) as tc:
        with (
            tc.tile_pool(name="const", bufs=1) as constp,
            tc.tile_pool(name="xt", bufs=2) as xtp,
            tc.tile_pool(name="acc_e", bufs=2) as accep,
            tc.tile_pool(name="acc_o", bufs=2) as accop,
            tc.tile_pool(name="cos", bufs=2) as cosp,
            tc.tile_pool(name="tmp", bufs=4) as tmpp,
            tc.tile_pool(name="sE", bufs=2) as sEp,
            tc.tile_pool(name="sO", bufs=2) as sOp,
            tc.tile_pool(name="amid", bufs=2) as amidp,
            tc.tile_pool(name="c1", bufs=2) as c1p,
            tc.tile_pool(name="outt", bufs=2) as outp,
            tc.tile_pool(name="dg", bufs=2) as dgp,
            tc.tile_pool(name="dps", bufs=3, space="PSUM") as dpsp,
            tc.tile_pool(name="wps", bufs=3, space="PSUM") as wpsp,
        ):
            accpools = {"acc_e": accep, "acc_o": accop}

            w1t = constp.tile([128, 3, NB, NB, 128], BF16)
            nc.sync.dma_start(w1t[:], w1t_d[:])
            w2t = constp.tile([128, 3, NB, NB, 128], BF16)
            nc.sync.dma_start(w2t[:], w2t_d[:])
            sc_t = constp.tile([128, NB, 64], F32)
            nc.sync.dma_start(sc_t[:], sc_d[:])

            def upconv_snake(b, src_tile, width, off, E, O):
                """One block's up-convs (both phases) + snake into E/O.
                off = stage scalar-column offset (0 or S2)."""
                for phase, dst in ((0, E), (1, O)):
                    wb = off + 6 * phase
                    tag = "acc_e" if phase == 0 else "acc_o"
                    accp = accpools[tag]
                    # tap 0 on ACT: acc = we0*x + inv2b
                    acc = accp.tile([128, NB, width], BF16, tag=tag)
                    nc.scalar.activation(
                        acc[:, b, :], src_tile[:, b, 0:width], AF.Identity,
                        bias=sc_t[:, b, off + 27:off + 28],
                        scale=sc_t[:, b, wb:wb + 1],
                    )
                    cur = acc
                    # taps 1-4 on DVE (STT)
                    for k in range(1, 5):
                        nxt = accp.tile([128, NB, width], BF16, tag=tag)
                        nc.vector.scalar_tensor_tensor(
                            nxt[:, b, :], src_tile[:, b, k:k + width],
                            sc_t[:, b, wb + k:wb + k + 1], cur[:, b, :],
                            ALU.mult, ALU.add,
                        )
                        cur = nxt
                    # tap 5 on Pool: mul(bcast) + add
                    pt = tmpp.tile([128, width], BF16, tag="tmp")
                    nc.gpsimd.tensor_mul(
                        pt[:, :], src_tile[:, b, 5:5 + width],
                        sc_t[:, b, wb + 5:wb + 6].broadcast_to([128, width]),
                    )
                    fin = accp.tile([128, NB, width], BF16, tag=tag)
                    nc.gpsimd.tensor_add(fin[:, b, :], pt[:, :], cur[:, b, :])
                    # cos on ACT
                    cost = cosp.tile([128, NB, width], BF16, tag="cos")
                    nc.scalar.activation(
                        cost[:, b, :], fin[:, b, :], AF.Sin,
                        bias=sc_t[:, b, off + 25:off + 26],
                        scale=sc_t[:, b, off + 24:off + 25],
                    )
                    # snake combine on Pool: dst = cos*(-inv2b) + acc
                    st = tmpp.tile([128, width], BF16, tag="tmp")
                    nc.gpsimd.tensor_mul(
                        st[:, :], cost[:, b, :],
                        sc_t[:, b, off + 26:off + 27].broadcast_to([128, width]),
                    )
                    nc.gpsimd.tensor_add(dst[:, b, :], st[:, :], fin[:, b, :])

            def downconv(b, E, O, width, dgt, dst):
                """12-tap two-phase down conv on PE -> dst (via ACT evict)."""
                for c0, n in _chunks(width):
                    ps = wpsp.tile([128, CHUNK], F32, tag="wps")
                    for r in range(6):
                        nc.tensor.matmul(
                            ps[:, :n], dgt[:, b * 12 + r, :],
                            O[:, b, c0 + r:c0 + r + n],
                            start=(r == 0), stop=False,
                        )
                    for r in range(6):
                        nc.tensor.matmul(
                            ps[:, :n], dgt[:, b * 12 + 6 + r, :],
                            E[:, b, c0 + r + 1:c0 + r + 1 + n],
                            start=False, stop=(r == 5),
                        )
                    nc.scalar.copy(dst[:, b, c0:c0 + n], ps[:, :n])

            for i in range(NT):
                t0 = i * L
                first, last = i == 0, i == NT - 1
                W1, s1 = L + 21, t0 - 8
                W2, s2 = L + 15, t0 - 6
                W3, s3 = L + 13, t0 - 5
                W4, s4 = L + 8, t0 - 3
                W5, s5 = L + 2, t0 - 1
                Wx, sx = L + 26, t0 - 10

                xt = xtp.tile([128, NB, Wx], BF16)
                nc.sync.dma_start(xt[:], xp_d[:, :, sx + PAD:sx + PAD + Wx])

                # stage 1 up + snake
                E1 = sEp.tile([128, NB, W1], BF16, tag="sE")
                O1 = sOp.tile([128, NB, W1], BF16, tag="sO")
                for b in range(NB):
                    upconv_snake(b, xt, W1, 0, E1, O1)
                if first:
                    nc.gpsimd.memset(E1[:, :, 0:1 - s1], 0.0)
                    nc.gpsimd.memset(O1[:, :, 0:1 - s1], 0.0)
                if last:
                    z = (T - 1) - s1
                    nc.gpsimd.memset(E1[:, :, z:W1], 0.0)
                    nc.gpsimd.memset(O1[:, :, z:W1], 0.0)

                # down1 -> a1
                dg1 = dgp.tile([128, NB * 12, 128], BF16, tag="dg")
                nc.sync.dma_start(dg1[:], diag_d[:, 0:NB * 12, :])
                a1 = amidp.tile([128, NB, W2], BF16, tag="amid")
                for b in range(NB):
                    downconv(b, E1, O1, W2, dg1, a1)
                if first:
                    nc.gpsimd.memset(a1[:, :, 0:0 - s2], 0.0)
                if last:
                    z = (T - 2) - s2
                    nc.gpsimd.memset(a1[:, :, z:W2], 0.0)

                # conv1 -> c1 (+bias1)
                c1 = c1p.tile([128, NB, W3], BF16, tag="c1")
                for o in range(NB):
                    for c0, n in _chunks(W3):
                        ps = dpsp.tile([128, CHUNK], F32, tag="dps")
                        for idx, (ib, k) in enumerate(
                            (ib, k) for ib in range(NB) for k in range(3)
                        ):
                            nc.tensor.matmul(
                                ps[:, :n], w1t[:, k, ib, o, :],
                                a1[:, ib, c0 + k:c0 + k + n],
                                start=(idx == 0), stop=(idx == 11),
                            )
                        nc.scalar.activation(
                            c1[:, o, c0:c0 + n], ps[:, :n], AF.Identity,
                            bias=sc_t[:, o, 56:57], scale=1.0,
                        )
                if first:
                    nc.gpsimd.memset(c1[:, :, 0:0 - s3], 0.0)
                if last:
                    z = (T - 2) - s3
                    nc.gpsimd.memset(c1[:, :, z:W3], 0.0)

                # stage 2 up + snake
                E2 = sEp.tile([128, NB, W4], BF16, tag="sE")
                O2 = sOp.tile([128, NB, W4], BF16, tag="sO")
                for b in range(NB):
                    upconv_snake(b, c1, W4, S2, E2, O2)
                if first:
                    nc.gpsimd.memset(E2[:, :, 0:1 - s4], 0.0)
                    nc.gpsimd.memset(O2[:, :, 0:1 - s4], 0.0)
                if last:
                    z = (T - 3) - s4
                    nc.gpsimd.memset(E2[:, :, z:W4], 0.0)
                    nc.gpsimd.memset(O2[:, :, z:W4], 0.0)

                # down2 -> a2
                dg2 = dgp.tile([128, NB * 12, 128], BF16, tag="dg")
                nc.sync.dma_start(dg2[:], diag_d[:, NB * 12:2 * NB * 12, :])
                a2 = amidp.tile([128, NB, W5], BF16, tag="amid")
                for b in range(NB):
                    downconv(b, E2, O2, W5, dg2, a2)
                if first:
                    nc.gpsimd.memset(a2[:, :, 0:0 - s5], 0.0)
                if last:
                    z = (T - 4) - s5
                    nc.gpsimd.memset(a2[:, :, z:W5], 0.0)

                # conv2 (+bias2 in evict) + residual -> out (bf16)
                Lo = min(L, TOUT - t0)
                outt = outp.tile([128, NB, L], BF16, tag="outt")
                for o in range(NB):
                    c0 = 0
                    while c0 < Lo:
                        n = min(CHUNK, Lo - c0)
                        ps = dpsp.tile([128, CHUNK], F32, tag="dps")
                        for idx, (ib, k) in enumerate(
                            (ib, k) for ib in range(NB) for k in range(3)
                        ):
                            nc.tensor.matmul(
                                ps[:, :n], w2t[:, k, ib, o, :],
                                a2[:, ib, c0 + k:c0 + k + n],
                                start=(idx == 0), stop=(idx == 11),
                            )
                        rt = tmpp.tile([128, CHUNK], BF16, tag="tmp")
                        nc.scalar.activation(
                            rt[:, :n], ps[:, :n], AF.Identity,
                            bias=sc_t[:, o, 57:58], scale=1.0,
                        )
                        nc.gpsimd.tensor_add(
                            outt[:, o, c0:c0 + n], rt[:, :n],
                            xt[:, o, 10 + c0:10 + c0 + n],
                        )
                        c0 += n
                nc.sync.dma_start(out_d[:, :, t0:t0 + Lo], outt[:, :, 0:Lo])
    nc.finalize()
    return nc


def _prep_host(x, up_w1, down_w1, alpha1, beta1, up_w2, down_w2, alpha2, beta2,
               c1_w, c1_b, c2_w, c2_b):
    bf = ml_dtypes.bfloat16
    B = x.shape[0]

    def dense_wt(w):
        out = np.empty((128, 3, NB, NB, 128), np.float32)
        wr = w.reshape(NB, 128, NB, 128, 3)  # o, co, i, ci, k
        out[:] = wr.transpose(3, 4, 2, 0, 1)  # (ci, k, i, o, co)
        return out.astype(bf)

    w1t = dense_wt(c1_w)
    w2t = dense_wt(c2_w)

    sc = np.zeros((128, NB, 64), np.float32)
    cidx = np.arange(C)
    for s, (up_w, down_w, alpha, beta) in enumerate(
        ((up_w1, down_w1, alpha1, beta1), (up_w2, down_w2, alpha2, beta2))
    ):
        off = s * S2
        a2v = 2.0 * np.exp(alpha)
        inv2b = 1.0 / (2.0 * np.exp(beta) + 1e-9)
        for b in range(NB):
            cs = cidx[b * 128:(b + 1) * 128]
            for k in range(6):
                sc[:, b, off + k] = up_w[2 * cs, k]
                sc[:, b, off + 6 + k] = up_w[2 * cs + 1, k]
                sc[:, b, off + 12 + k] = down_w[cs, 2 * k]
                sc[:, b, off + 18 + k] = down_w[cs, 2 * k + 1]
            sc[:, b, off + 24] = a2v[cs]
            sc[:, b, off + 25] = np.pi / 2 - a2v[cs] * inv2b[cs]
            sc[:, b, off + 26] = -inv2b[cs]
            sc[:, b, off + 27] = inv2b[cs]
    for b in range(NB):
        cs = cidx[b * 128:(b + 1) * 128]
        sc[:, b, 56] = c1_b[cs]
        sc[:, b, 57] = c2_b[cs]

    diag = np.zeros((128, 2 * NB * 12, 128), np.float32)
    for s, down_w in enumerate((down_w1, down_w2)):
        for b in range(NB):
            cs = cidx[b * 128:(b + 1) * 128]
            for r in range(6):
                i0 = s * NB * 12 + b * 12
                diag[np.arange(128), i0 + r, np.arange(128)] = down_w[cs, 2 * r]
                diag[np.arange(128), i0 + 6 + r, np.arange(128)] = down_w[cs, 2 * r + 1]
    diag = diag.astype(bf)

    in_maps = []
    for bi in range(B):
        xpad = np.zeros((C, T + 2 * PAD), np.float32)
        xpad[:, PAD:PAD + T] = x[bi]
        xp = np.ascontiguousarray(
            xpad.reshape(NB, 128, T + 2 * PAD).transpose(1, 0, 2)
        ).astype(bf)
        in_maps.append({
            "xp": xp, "w1t": w1t, "w2t": w2t, "sc": sc.astype(np.float32),
            "diag": diag,
        })
    return in_maps


_NC_CACHE = None


def _install_profile_hook():
    import types

    try:
        from antenv.axon_hooks import get_axon_ntff_profile_hook  # noqa: F401
        return
    except ImportError:
        pass
    try:
        import antenv
        mod = types.ModuleType("antenv.axon_hooks")
        _state = {"hook": None}
        mod.set_axon_ntff_profile_hook = lambda h: _state.__setitem__("hook", h)
        mod.get_axon_ntff_profile_hook = lambda: _state["hook"]
        sys.modules["antenv.axon_hooks"] = mod
        antenv.axon_hooks = mod
        if "/root/.axon_site" not in sys.path:
            sys.path.insert(0, "/root/.axon_site")
        from trn_agent_boot.trn_boot import _ntff_profile_via_ctypes
        mod.set_axon_ntff_profile_hook(
            _ntff_profile_via_ctypes("/opt/axon/libaxon_pjrt.so"))
    except Exception as e:
        print(f"profile hook install failed: {e}")


def kernel(**inputs):
    global _NC_CACHE, LAST_EXEC_NS, LAST_PROFILE
    import os

    args = {k: np.asarray(v) for k, v in inputs.items()}
    in_maps = _prep_host(**args)
    if _NC_CACHE is None:
        _NC_CACHE = build_graph()
    nc = _NC_CACHE
    trace = bool(os.environ.get("KERNEL_TRACE"))
    kw = {}
    if trace:
        _install_profile_hook()
        kw["tmpdir"] = os.environ.get("KERNEL_TRACE_DIR", "/tmp/ktrace")
        os.makedirs(kw["tmpdir"], exist_ok=True)
    res = run_bass_kernel_spmd(
        nc, in_maps, core_ids=list(range(8)), trace=trace, **kw,
    )
    LAST_EXEC_NS = res.exec_time_ns
    LAST_PROFILE = res.profile_json
    B = len(in_maps)
    out = np.empty((B, C, TOUT), np.float32)
    for bi in range(B):
        o = np.asarray(res.results[bi]["out"]).astype(np.float32)
        out[bi] = o.transpose(1, 0, 2).reshape(C, TOUT)
    return out
